# revision 14
# baseline (speedup 1.0000x reference)
"""Trainium2 Bass kernel for ContrastivePuzzleLoss.

Reference math (per batch b):
    f = features / max(||features||_2, 1e-12)           (L2 norm over D)
    sim = (f @ f.T) / T,  off-diag only
    pos_mask[i,j] = (pos_i == pos_j), off-diag only
    pos_s = sum_j sim*mask + eps ; neg_s = sum_j sim*(1-mask) + eps
    loss = mean softplus(neg_s - pos_s)

Key identity used here: eps cancels and
    neg_s - pos_s = rowsum_offdiag - 2*possum_offdiag
                  = ((f_i . S) - 2*(sum_j delta_ij f_i.f_j) + 1) / T
with S = sum_j f_j. Per-core work (pure data parallel over batch):
  - load transposed features fT [D,N] (bf16)
  - norms via ACT Square + ones-matmul (PE), r = exp(-0.5*ln(norm^2+1e-24))
  - normalize: one fused DVE scalar_tensor_tensor pass per k-tile
    (out = fT * r_bcast, accum_out = S chunk)
  - sim rows via PE matmul (bf16), with f_i.S folded in as one extra
    psum column (matvec against S)
  - possum in ONE fused DVE pass per row tile:
    (pos_bcast == pos_i) * sim, accumulated along the row
  - softplus via ACT: relu(x) + ln(1 + exp(-|x|))
  - per-core scalar = sum of losses; host sums cores and divides.
"""

import json

import numpy as np
import ml_dtypes

import concourse.bass as bass
import concourse.tile as tile
import concourse.mybir as mybir
from concourse.bass_utils import run_bass_kernel_spmd

B, N, D = 64, 576, 768
NCORES = 8
BPC = B // NCORES          # batches per core
KT = D // 128              # 6 contraction tiles
MT = (N + 127) // 128      # 5 row tiles (last has 64 rows)
TEMP = 0.07

F32 = mybir.dt.float32
F32R = mybir.dt.float32r
BF16 = mybir.dt.bfloat16
AF = mybir.ActivationFunctionType
ALU = mybir.AluOpType


def _legalize_sync_json(raw: bytes) -> bytes:
    """The hardware ISA has ONE sync-wait slot per instruction, and this
    walrus build refuses multi-wait instructions ("Too many sync wait
    commands"). Split extra waits onto injected single-wait Drain
    instructions on the same engine, preceding the original."""
    d = json.loads(raw)
    nid = [0]
    for fn in d["functions"]:
        for blk in fn["blocks"]:
            out = []
            for ins in blk["instructions"]:
                si = ins.get("sync_info") or {}
                w = si.get("on_wait") or []
                if len(w) > 1:
                    for extra in w[:-1]:
                        nid[0] += 1
                        out.append(
                            {
                                "debug": ins.get("debug", 0),
                                "engine": ins["engine"],
                                "name": f"I-WSPLIT-{nid[0]}",
                                "opcode": "Drain",
                                "ins": [],
                                "outs": [],
                                "sync_info": {"on_wait": [extra], "on_update": []},
                            }
                        )
                    si["on_wait"] = [w[-1]]
                out.append(ins)
            blk["instructions"] = out
    return json.dumps(d).encode()


def build_nc(bpc=BPC):
    nc = bass.Bass()

    ft_d = nc.dram_tensor("ft", [bpc, KT, 128, N], BF16, kind="ExternalInput")
    posf_d = nc.dram_tensor("posf", [bpc, 1, N], F32, kind="ExternalInput")
    pospack_d = nc.dram_tensor("pospack", [bpc, 128, MT], F32, kind="ExternalInput")
    vmask_d = nc.dram_tensor("vmask", [128, MT], F32, kind="ExternalInput")
    out_d = nc.dram_tensor("out", [1, 1], F32, kind="ExternalOutput")

    with tile.TileContext(nc) as tc:
        with (
            tc.tile_pool(name="ftp", bufs=2) as ftp,            # raw fT tiles
            tc.tile_pool(name="ftnp", bufs=2) as ftnp,          # normalized fT
            tc.tile_pool(name="sqp", bufs=7) as sqp,            # squares
            tc.tile_pool(name="bcp", bufs=2) as bcp,            # broadcast tiles
            tc.tile_pool(name="smallp", bufs=3) as smallp,      # small per-batch
            tc.tile_pool(name="scrp", bufs=2) as scrp,          # STT scratch out
            tc.tile_pool(name="singles", bufs=1) as singles,
            tc.tile_pool(name="psb", bufs=3, space=bass.MemorySpace.PSUM) as psb,
            tc.tile_pool(name="psn", bufs=1, space=bass.MemorySpace.PSUM) as psn,
            tc.tile_pool(name="drp", bufs=2, space="DRAM") as drp,
        ):
            ones_bf = singles.tile([128, 1], BF16)
            nc.vector.memset(ones_bf, 1.0)
            ones_f32 = singles.tile([128, 1], F32)
            nc.vector.memset(ones_f32, 1.0)
            vmask_t = singles.tile([128, MT], F32)
            nc.sync.dma_start(out=vmask_t, in_=vmask_d[:])
            eps_t = singles.tile([1, 1], F32)
            nc.vector.memset(eps_t, 1e-24)
            acc = singles.tile([128, MT], F32)
            nc.vector.memset(acc, 0.0)

            for b in range(bpc):
                # ---- load transposed features (one DMA, permuted AP) ----
                ft_t = ftp.tile([128, KT, N], BF16)
                nc.sync.dma_start(
                    out=ft_t, in_=ft_d[b].rearrange("k p n -> p k n")
                )

                # ---- squares (ACT) + norms via ones-matmul (PE) ----
                psum_n = psn.tile([1, N], F32, tag="pn")
                for k in range(KT):
                    sq_k = sqp.tile([128, N], BF16, tag="sq")
                    nc.scalar.activation(sq_k, ft_t[:, k, :], AF.Square)
                    nc.tensor.matmul(
                        psum_n[:, 0:512], ones_bf, sq_k[:, 0:512],
                        start=(k == 0), stop=(k == KT - 1),
                    )
                    nc.tensor.matmul(
                        psum_n[:, 512:N], ones_bf, sq_k[:, 512:N],
                        start=(k == 0), stop=(k == KT - 1),
                    )

                # ---- r = exp(-0.5 * ln(norm^2 + 1e-24)) ----
                ln_row = smallp.tile([1, N], F32, tag="lnr")
                nc.scalar.activation(ln_row, psum_n[:, :], AF.Ln, bias=eps_t)
                r_row = smallp.tile([1, N], BF16, tag="rr")
                nc.scalar.activation(r_row, ln_row, AF.Exp, scale=-0.5)

                # round-trip through DRAM to broadcast r along partitions
                r_dram = drp.tile([1, N], BF16)
                nc.sync.dma_start(out=r_dram, in_=r_row)
                r_bc = bcp.tile([128, N], BF16, tag="rbc")
                nc.gpsimd.dma_start(out=r_bc, in_=r_dram[:].to_broadcast([128, N]))

                pos_bc = bcp.tile([128, N], F32, tag="pbc")
                nc.gpsimd.dma_start(
                    out=pos_bc, in_=posf_d[b].to_broadcast([128, N])
                )
                pos_pack = smallp.tile([128, MT], F32, tag="ppk")
                nc.sync.dma_start(out=pos_pack, in_=pospack_d[b])

                # ---- normalize + accumulate S (one fused DVE pass per k) ----
                ftn_t = ftnp.tile([128, KT, N], BF16)
                s_sum = smallp.tile([128, KT], BF16, tag="ssum")
                for k in range(KT):
                    nc.vector.scalar_tensor_tensor(
                        out=ftn_t[:, k, :],
                        in0=ft_t[:, k, :],
                        scalar=1.0,
                        in1=r_bc,
                        op0=ALU.mult,
                        op1=ALU.mult,
                        accum_out=s_sum[:, k : k + 1],
                    )

                # ---- per row-tile: sim matmul + matvec + fused possum ----
                ps_pack = smallp.tile([128, MT], F32, tag="pspk")
                arg_pack = smallp.tile([128, MT], F32, tag="argpk")
                nc.vector.memset(arg_pack, 0.0)
                for m in range(MT):
                    mm = min(128, N - m * 128)
                    psum_s = psb.tile([128, 640], F32, tag="ps")
                    regions = [
                        (psum_s[:mm, 0:512], lambda k: ftn_t[:, k, 0:512]),
                        (psum_s[:mm, 512:N], lambda k: ftn_t[:, k, 512:N]),
                        (psum_s[:mm, N : N + 1], lambda k: s_sum[:, k : k + 1]),
                    ]
                    for out_r, rhs_fn in regions:
                        for k in range(KT):
                            nc.tensor.matmul(
                                out_r,
                                ftn_t[:, k, m * 128 : m * 128 + mm],
                                rhs_fn(k),
                                start=(k == 0), stop=(k == KT - 1),
                            )
                    # possum: (pos_bc == pos_i) * sim, accumulated along row
                    scr = scrp.tile([128, N], BF16, tag="scr")
                    nc.vector.scalar_tensor_tensor(
                        out=scr[:mm, :],
                        in0=pos_bc[:mm, :],
                        scalar=pos_pack[:mm, m : m + 1],
                        in1=psum_s[:mm, 0:N],
                        op0=ALU.is_equal,
                        op1=ALU.mult,
                        accum_out=ps_pack[:mm, m : m + 1],
                    )
                    # arg = t - 2*ps  (t = f_i . S, in psum col N)
                    nc.vector.scalar_tensor_tensor(
                        out=arg_pack[:mm, m : m + 1],
                        in0=ps_pack[:mm, m : m + 1],
                        scalar=-2.0,
                        in1=psum_s[:mm, N : N + 1],
                        op0=ALU.mult,
                        op1=ALU.add,
                    )

                # ---- y = (arg + 1)/T ; softplus = relu(y) + ln(1+exp(-|y|)) ----
                y = smallp.tile([128, MT], F32, tag="y")
                nc.vector.tensor_scalar(
                    out=y, in0=arg_pack, scalar1=1.0, scalar2=1.0 / TEMP,
                    op0=ALU.add, op1=ALU.mult,
                )
                ab = smallp.tile([128, MT], F32, tag="ab")
                nc.scalar.activation(ab, y, AF.Abs)
                ex = smallp.tile([128, MT], F32, tag="ex")
                nc.scalar.activation(ex, ab, AF.Exp, scale=-1.0)
                ln1p = smallp.tile([128, MT], F32, tag="ln1p")
                nc.scalar.activation(ln1p, ex, AF.Ln, bias=1.0)
                mx = smallp.tile([128, MT], F32, tag="mx")
                nc.scalar.activation(mx, y, AF.Relu)
                # loss = (mx + ln1p) * vmask, accumulated into acc
                sp = smallp.tile([128, MT], F32, tag="sp")
                nc.vector.tensor_add(sp, mx, ln1p)
                spm = smallp.tile([128, MT], F32, tag="spm")
                nc.vector.tensor_mul(spm, sp, vmask_t)
                nc.vector.tensor_add(acc, acc, spm)

            # ---- final: sum acc over all entries -> scalar ----
            red = singles.tile([128, 1], F32)
            nc.vector.reduce_sum(red, acc, axis=mybir.AxisListType.X)
            psum_f = psn.tile([1, 1], F32, tag="pn")
            nc.tensor.matmul(psum_f, ones_f32, red)
            out_sb = singles.tile([1, 1], F32)
            nc.scalar.copy(out_sb, psum_f)
            nc.sync.dma_start(out=out_d[:], in_=out_sb)

    nc.finalize()
    fixed = _legalize_sync_json(bytes(nc.to_json_bytes()))
    nc.to_json_bytes = lambda: fixed  # instance override: walrus-legal BIR
    return nc


def _prep_inputs(features, positions, bpc_total=B):
    feats = np.asarray(features, dtype=np.float32).reshape(B, N, D)
    pos = np.asarray(positions).astype(np.float32)  # values < 2^24, exact
    fT = np.ascontiguousarray(feats.transpose(0, 2, 1))  # [B, D, N]
    fT = fT.reshape(B, KT, 128, N).astype(ml_dtypes.bfloat16)
    pos_pack = np.full((B, 128, MT), -1.0, dtype=np.float32)
    for m in range(MT):
        lo = m * 128
        hi = min(N, lo + 128)
        pos_pack[:, : hi - lo, m] = pos[:, lo:hi]
    vmask = np.zeros((128, MT), dtype=np.float32)
    for m in range(MT):
        lo = m * 128
        hi = min(N, lo + 128)
        vmask[: hi - lo, m] = 1.0
    return fT, pos.reshape(B, 1, N), pos_pack, vmask


def _install_ntff_hook_shim():
    """This image's boot skipped installing the axon NTFF profile hook
    (no antenv.axon_hooks module). Recreate it so trace=True works."""
    import sys as _sys
    import types as _types

    if "antenv.axon_hooks" in _sys.modules:
        return
    try:
        from trn_agent_boot.trn_boot import _ntff_profile_via_ctypes

        hook = _ntff_profile_via_ctypes("/opt/axon/libaxon_pjrt.so")
    except Exception:
        return
    import antenv as _antenv

    mod = _types.ModuleType("antenv.axon_hooks")
    mod.get_axon_ntff_profile_hook = lambda: hook
    mod.set_axon_ntff_profile_hook = lambda h: None
    _sys.modules["antenv.axon_hooks"] = mod
    _antenv.axon_hooks = mod


_install_ntff_hook_shim()

_NC_CACHE = {}
LAST_RESULTS = None  # BassKernelResults of the most recent run (for profiling)


def kernel(features, positions, _trace=False):
    global LAST_RESULTS
    fT, posf, pos_pack, vmask = _prep_inputs(features, positions)
    if BPC not in _NC_CACHE:
        _NC_CACHE[BPC] = build_nc(BPC)
    nc = _NC_CACHE[BPC]
    in_maps = []
    for c in range(NCORES):
        s = slice(c * BPC, (c + 1) * BPC)
        in_maps.append(
            {
                "ft": np.ascontiguousarray(fT[s]),
                "posf": np.ascontiguousarray(posf[s]),
                "pospack": np.ascontiguousarray(pos_pack[s]),
                "vmask": vmask,
            }
        )
    res = run_bass_kernel_spmd(
        nc, in_maps, core_ids=list(range(NCORES)), trace=_trace
    )
    LAST_RESULTS = res
    total = sum(float(r["out"][0, 0]) for r in res.results)
    return np.float32(total / (B * N))


# revision 19
# speedup vs baseline: 1.0811x; 1.0811x over previous
"""Trainium2 Bass kernel for ContrastivePuzzleLoss.

Reference math (per batch b):
    f = features / max(||features||_2, 1e-12)           (L2 norm over D)
    sim = (f @ f.T) / T,  off-diag only
    pos_mask[i,j] = (pos_i == pos_j), off-diag only
    pos_s = sum_j sim*mask + eps ; neg_s = sum_j sim*(1-mask) + eps
    loss = mean softplus(neg_s - pos_s)

Key identity used here: eps cancels and
    neg_s - pos_s = rowsum_offdiag - 2*possum_offdiag
                  = ((f_i . S) - 2*(sum_j delta_ij f_i.f_j) + 1) / T
with S = sum_j f_j. Per-core work (pure data parallel over batch):
  - load transposed features fT [D,N] (bf16)
  - norms via ACT Square + ones-matmul (PE), r = exp(-0.5*ln(norm^2+1e-24))
  - normalize: one fused DVE scalar_tensor_tensor pass per k-tile
    (out = fT * r_bcast, accum_out = S chunk)
  - sim rows via PE matmul (bf16), with f_i.S folded in as one extra
    psum column (matvec against S)
  - possum in ONE fused DVE pass per row tile:
    (pos_bcast == pos_i) * sim, accumulated along the row
  - softplus via ACT: relu(x) + ln(1 + exp(-|x|))
  - per-core scalar = sum of losses; host sums cores and divides.
"""

import json

import numpy as np
import ml_dtypes

import concourse.bass as bass
import concourse.tile as tile
import concourse.mybir as mybir
from concourse.bass_utils import run_bass_kernel_spmd

B, N, D = 64, 576, 768
NCORES = 8
BPC = B // NCORES          # batches per core
KT = D // 128              # 6 contraction tiles
MT = (N + 127) // 128      # 5 row tiles (last has 64 rows)
TEMP = 0.07

F32 = mybir.dt.float32
F32R = mybir.dt.float32r
BF16 = mybir.dt.bfloat16
AF = mybir.ActivationFunctionType
ALU = mybir.AluOpType


def _legalize_sync_json(raw: bytes) -> bytes:
    """The hardware ISA has ONE sync-wait slot per instruction, and this
    walrus build refuses multi-wait instructions ("Too many sync wait
    commands"). Split extra waits onto injected single-wait Drain
    instructions on the same engine, preceding the original."""
    d = json.loads(raw)
    nid = [0]

    def mk_drain(ins, wait):
        nid[0] += 1
        return {
            "debug": ins.get("debug", 0),
            "engine": ins["engine"],
            "name": f"I-WSPLIT-{nid[0]}",
            "opcode": "Drain",
            "ins": [],
            "outs": [],
            "sync_info": {"on_wait": [wait], "on_update": []},
        }

    for fn in d["functions"]:
        for blk in fn["blocks"]:
            out = []
            for ins in blk["instructions"]:
                si = ins.get("sync_info") or {}
                w = si.get("on_wait") or []
                if len(w) <= 1:
                    out.append(ins)
                    continue
                extras = w[:-1]
                si["on_wait"] = [w[-1]]
                # A PE Matmult is normally preceded by its Ldweights with a
                # free wait slot — park one wait there (no pipeline flush).
                prev = out[-1] if out else None
                if (
                    ins["opcode"] == "Matmult"
                    and prev is not None
                    and prev.get("opcode") == "Ldweights"
                    and prev.get("engine") == ins["engine"]
                    and not ((prev.get("sync_info") or {}).get("on_wait") or [])
                ):
                    psi = prev.setdefault("sync_info", {})
                    psi["on_wait"] = [extras.pop()]
                # Remaining extras ride single-wait Drains inserted before
                # the instruction (and before its Ldweights, if any).
                ipos = len(out)
                if (
                    prev is not None
                    and prev.get("opcode") == "Ldweights"
                    and prev.get("engine") == ins["engine"]
                ):
                    ipos -= 1
                for extra in extras:
                    out.insert(ipos, mk_drain(ins, extra))
                out.append(ins)
            blk["instructions"] = out
    return json.dumps(d).encode()


def build_nc(bpc=BPC):
    nc = bass.Bass()

    ft_d = nc.dram_tensor("ft", [bpc, KT, 128, N], BF16, kind="ExternalInput")
    posf_d = nc.dram_tensor("posf", [bpc, 1, N], F32, kind="ExternalInput")
    pospack_d = nc.dram_tensor("pospack", [bpc, 128, MT], F32, kind="ExternalInput")
    vmask_d = nc.dram_tensor("vmask", [128, MT], F32, kind="ExternalInput")
    out_d = nc.dram_tensor("out", [1, 1], F32, kind="ExternalOutput")

    with tile.TileContext(nc) as tc:
        with (
            tc.tile_pool(name="ftp", bufs=2) as ftp,            # raw fT tiles
            tc.tile_pool(name="ftnp", bufs=2) as ftnp,          # normalized fT
            tc.tile_pool(name="sqp", bufs=7) as sqp,            # squares
            tc.tile_pool(name="bcp", bufs=2) as bcp,            # broadcast tiles
            tc.tile_pool(name="smallp", bufs=3) as smallp,      # small per-batch
            tc.tile_pool(name="scrp", bufs=2) as scrp,          # STT scratch out
            tc.tile_pool(name="singles", bufs=1) as singles,
            tc.tile_pool(name="psb", bufs=3, space=bass.MemorySpace.PSUM) as psb,
            tc.tile_pool(name="psn", bufs=1, space=bass.MemorySpace.PSUM) as psn,
            tc.tile_pool(name="drp", bufs=2, space="DRAM") as drp,
        ):
            ones_bf = singles.tile([128, 1], BF16)
            nc.vector.memset(ones_bf, 1.0)
            ones_f32 = singles.tile([128, 1], F32)
            nc.vector.memset(ones_f32, 1.0)
            vmask_t = singles.tile([128, MT], F32)
            nc.sync.dma_start(out=vmask_t, in_=vmask_d[:])
            eps_t = singles.tile([1, 1], F32)
            nc.vector.memset(eps_t, 1e-24)
            acc = singles.tile([128, MT], F32)
            nc.vector.memset(acc, 0.0)

            for b in range(bpc):
                # ---- load transposed features (one DMA, permuted AP) ----
                ft_t = ftp.tile([128, KT, N], BF16)
                nc.sync.dma_start(
                    out=ft_t, in_=ft_d[b].rearrange("k p n -> p k n")
                )

                # ---- squares (ACT) + norms via ones-matmul (PE) ----
                psum_n = psn.tile([1, N], F32, tag="pn")
                for k in range(KT):
                    sq_k = sqp.tile([128, N], BF16, tag="sq")
                    nc.scalar.activation(sq_k, ft_t[:, k, :], AF.Square)
                    nc.tensor.matmul(
                        psum_n[:, 0:512], ones_bf, sq_k[:, 0:512],
                        start=(k == 0), stop=(k == KT - 1),
                    )
                    nc.tensor.matmul(
                        psum_n[:, 512:N], ones_bf, sq_k[:, 512:N],
                        start=(k == 0), stop=(k == KT - 1),
                    )

                # ---- r = exp(-0.5 * ln(norm^2 + 1e-24)) ----
                ln_row = smallp.tile([1, N], F32, tag="lnr")
                nc.scalar.activation(ln_row, psum_n[:, :], AF.Ln, bias=eps_t)
                r_row = smallp.tile([1, N], BF16, tag="rr")
                nc.scalar.activation(r_row, ln_row, AF.Exp, scale=-0.5)

                # round-trip through DRAM to broadcast r along partitions
                r_dram = drp.tile([1, N], BF16)
                nc.sync.dma_start(out=r_dram, in_=r_row)
                r_bc = bcp.tile([128, N], BF16, tag="rbc")
                nc.sync.dma_start(out=r_bc, in_=r_dram[:].to_broadcast([128, N]))

                pos_bc = bcp.tile([128, N], F32, tag="pbc")
                nc.sync.dma_start(
                    out=pos_bc, in_=posf_d[b].to_broadcast([128, N])
                )
                pos_pack = smallp.tile([128, MT], F32, tag="ppk")
                nc.sync.dma_start(out=pos_pack, in_=pospack_d[b])

                # ---- normalize + accumulate S (one fused DVE pass per k) ----
                # S chunk lands in column N of the same tile, so the sim
                # matmul streams it as one extra rhs column (t = f_i . S
                # appears in psum column N for free - no matvec matmuls).
                ftn_t = ftnp.tile([128, KT, N + 2], BF16)
                for k in range(KT):
                    nc.vector.scalar_tensor_tensor(
                        out=ftn_t[:, k, 0:N],
                        in0=ft_t[:, k, :],
                        scalar=1.0,
                        in1=r_bc,
                        op0=ALU.mult,
                        op1=ALU.mult,
                        accum_out=ftn_t[:, k, N : N + 1],
                    )

                # ---- per row-tile: sim matmul + matvec + fused possum ----
                ps_pack = smallp.tile([128, MT], F32, tag="pspk")
                arg_pack = smallp.tile([128, MT], F32, tag="argpk")
                nc.vector.memset(arg_pack, 0.0)
                for m in range(MT):
                    mm = min(128, N - m * 128)
                    psum_s = psb.tile([128, 640], F32, tag="ps")
                    regions = [
                        (psum_s[:mm, 0:512], lambda k: ftn_t[:, k, 0:512]),
                        (psum_s[:mm, 512 : N + 1], lambda k: ftn_t[:, k, 512 : N + 1]),
                    ]
                    for out_r, rhs_fn in regions:
                        for k in range(KT):
                            nc.tensor.matmul(
                                out_r,
                                ftn_t[:, k, m * 128 : m * 128 + mm],
                                rhs_fn(k),
                                start=(k == 0), stop=(k == KT - 1),
                            )
                    # possum: (pos_bc == pos_i) * sim, accumulated along row
                    scr = scrp.tile([128, N], BF16, tag="scr")
                    nc.vector.scalar_tensor_tensor(
                        out=scr[:mm, :],
                        in0=pos_bc[:mm, :],
                        scalar=pos_pack[:mm, m : m + 1],
                        in1=psum_s[:mm, 0:N],
                        op0=ALU.is_equal,
                        op1=ALU.mult,
                        accum_out=ps_pack[:mm, m : m + 1],
                    )
                    # arg = t - 2*ps  (t = f_i . S, in psum col N)
                    nc.vector.scalar_tensor_tensor(
                        out=arg_pack[:mm, m : m + 1],
                        in0=ps_pack[:mm, m : m + 1],
                        scalar=-2.0,
                        in1=psum_s[:mm, N : N + 1],
                        op0=ALU.mult,
                        op1=ALU.add,
                    )

                # ---- y = (arg + 1)/T ; softplus = relu(y) + ln(1+exp(-|y|)) ----
                y = smallp.tile([128, MT], F32, tag="y")
                nc.vector.tensor_scalar(
                    out=y, in0=arg_pack, scalar1=1.0, scalar2=1.0 / TEMP,
                    op0=ALU.add, op1=ALU.mult,
                )
                ab = smallp.tile([128, MT], F32, tag="ab")
                nc.scalar.activation(ab, y, AF.Abs)
                ex = smallp.tile([128, MT], F32, tag="ex")
                nc.scalar.activation(ex, ab, AF.Exp, scale=-1.0)
                ln1p = smallp.tile([128, MT], F32, tag="ln1p")
                nc.scalar.activation(ln1p, ex, AF.Ln, bias=1.0)
                mx = smallp.tile([128, MT], F32, tag="mx")
                nc.scalar.activation(mx, y, AF.Relu)
                # loss = (mx + ln1p) * vmask, accumulated into acc
                sp = smallp.tile([128, MT], F32, tag="sp")
                nc.vector.tensor_add(sp, mx, ln1p)
                spm = smallp.tile([128, MT], F32, tag="spm")
                nc.vector.tensor_mul(spm, sp, vmask_t)
                nc.vector.tensor_add(acc, acc, spm)

            # ---- final: sum acc over all entries -> scalar ----
            red = singles.tile([128, 1], F32)
            nc.vector.reduce_sum(red, acc, axis=mybir.AxisListType.X)
            psum_f = psn.tile([1, 1], F32, tag="pn")
            nc.tensor.matmul(psum_f, ones_f32, red)
            out_sb = singles.tile([1, 1], F32)
            nc.scalar.copy(out_sb, psum_f)
            nc.sync.dma_start(out=out_d[:], in_=out_sb)

    nc.finalize()
    fixed = _legalize_sync_json(bytes(nc.to_json_bytes()))
    nc.to_json_bytes = lambda: fixed  # instance override: walrus-legal BIR
    return nc


def _prep_inputs(features, positions, bpc_total=B):
    feats = np.asarray(features, dtype=np.float32).reshape(B, N, D)
    pos = np.asarray(positions).astype(np.float32)  # values < 2^24, exact
    fT = np.ascontiguousarray(feats.transpose(0, 2, 1))  # [B, D, N]
    fT = fT.reshape(B, KT, 128, N).astype(ml_dtypes.bfloat16)
    pos_pack = np.full((B, 128, MT), -1.0, dtype=np.float32)
    for m in range(MT):
        lo = m * 128
        hi = min(N, lo + 128)
        pos_pack[:, : hi - lo, m] = pos[:, lo:hi]
    vmask = np.zeros((128, MT), dtype=np.float32)
    for m in range(MT):
        lo = m * 128
        hi = min(N, lo + 128)
        vmask[: hi - lo, m] = 1.0
    return fT, pos.reshape(B, 1, N), pos_pack, vmask


def _install_ntff_hook_shim():
    """This image's boot skipped installing the axon NTFF profile hook
    (no antenv.axon_hooks module). Recreate it so trace=True works."""
    import sys as _sys
    import types as _types

    if "antenv.axon_hooks" in _sys.modules:
        return
    try:
        from trn_agent_boot.trn_boot import _ntff_profile_via_ctypes

        hook = _ntff_profile_via_ctypes("/opt/axon/libaxon_pjrt.so")
    except Exception:
        return
    import antenv as _antenv

    mod = _types.ModuleType("antenv.axon_hooks")
    mod.get_axon_ntff_profile_hook = lambda: hook
    mod.set_axon_ntff_profile_hook = lambda h: None
    _sys.modules["antenv.axon_hooks"] = mod
    _antenv.axon_hooks = mod


_install_ntff_hook_shim()

_NC_CACHE = {}
LAST_RESULTS = None  # BassKernelResults of the most recent run (for profiling)


def kernel(features, positions, _trace=False):
    global LAST_RESULTS
    fT, posf, pos_pack, vmask = _prep_inputs(features, positions)
    if BPC not in _NC_CACHE:
        _NC_CACHE[BPC] = build_nc(BPC)
    nc = _NC_CACHE[BPC]
    in_maps = []
    for c in range(NCORES):
        s = slice(c * BPC, (c + 1) * BPC)
        in_maps.append(
            {
                "ft": np.ascontiguousarray(fT[s]),
                "posf": np.ascontiguousarray(posf[s]),
                "pospack": np.ascontiguousarray(pos_pack[s]),
                "vmask": vmask,
            }
        )
    res = run_bass_kernel_spmd(
        nc, in_maps, core_ids=list(range(NCORES)), trace=_trace
    )
    LAST_RESULTS = res
    total = sum(float(r["out"][0, 0]) for r in res.results)
    return np.float32(total / (B * N))


# revision 21
# speedup vs baseline: 1.1371x; 1.0518x over previous
"""Trainium2 Bass kernel for ContrastivePuzzleLoss.

Reference math (per batch b):
    f = features / max(||features||_2, 1e-12)           (L2 norm over D)
    sim = (f @ f.T) / T,  off-diag only
    pos_mask[i,j] = (pos_i == pos_j), off-diag only
    pos_s = sum_j sim*mask + eps ; neg_s = sum_j sim*(1-mask) + eps
    loss = mean softplus(neg_s - pos_s)

Key identity used here: eps cancels and
    neg_s - pos_s = rowsum_offdiag - 2*possum_offdiag
                  = ((f_i . S) - 2*(sum_j delta_ij f_i.f_j) + 1) / T
with S = sum_j f_j. Per-core work (pure data parallel over batch):
  - load transposed features fT [D,N] (bf16)
  - norms via ACT Square + ones-matmul (PE), r = exp(-0.5*ln(norm^2+1e-24))
  - normalize: one fused DVE scalar_tensor_tensor pass per k-tile
    (out = fT * r_bcast, accum_out = S chunk)
  - sim rows via PE matmul (bf16), with f_i.S folded in as one extra
    psum column (matvec against S)
  - possum in ONE fused DVE pass per row tile:
    (pos_bcast == pos_i) * sim, accumulated along the row
  - softplus via ACT: relu(x) + ln(1 + exp(-|x|))
  - per-core scalar = sum of losses; host sums cores and divides.
"""

import json

import numpy as np
import ml_dtypes

import concourse.bass as bass
import concourse.tile as tile
import concourse.mybir as mybir
from concourse.bass_utils import run_bass_kernel_spmd

B, N, D = 64, 576, 768
NCORES = 8
BPC = B // NCORES          # batches per core
KT = D // 128              # 6 contraction tiles
MT = (N + 127) // 128      # 5 row tiles (last has 64 rows)
TEMP = 0.07

F32 = mybir.dt.float32
F32R = mybir.dt.float32r
BF16 = mybir.dt.bfloat16
AF = mybir.ActivationFunctionType
ALU = mybir.AluOpType


def _legalize_sync_json(raw: bytes) -> bytes:
    """The hardware ISA has ONE sync-wait slot per instruction, and this
    walrus build refuses multi-wait instructions ("Too many sync wait
    commands"). Split extra waits onto injected single-wait Drain
    instructions on the same engine, preceding the original."""
    d = json.loads(raw)
    nid = [0]

    def mk_drain(ins, wait):
        nid[0] += 1
        return {
            "debug": ins.get("debug", 0),
            "engine": ins["engine"],
            "name": f"I-WSPLIT-{nid[0]}",
            "opcode": "Drain",
            "ins": [],
            "outs": [],
            "sync_info": {"on_wait": [wait], "on_update": []},
        }

    for fn in d["functions"]:
        for blk in fn["blocks"]:
            out = []
            for ins in blk["instructions"]:
                si = ins.get("sync_info") or {}
                w = si.get("on_wait") or []
                if len(w) <= 1:
                    out.append(ins)
                    continue
                extras = w[:-1]
                si["on_wait"] = [w[-1]]
                # A PE Matmult is normally preceded by its Ldweights with a
                # free wait slot — park one wait there (no pipeline flush).
                prev = out[-1] if out else None
                if (
                    ins["opcode"] == "Matmult"
                    and prev is not None
                    and prev.get("opcode") == "Ldweights"
                    and prev.get("engine") == ins["engine"]
                    and not ((prev.get("sync_info") or {}).get("on_wait") or [])
                ):
                    psi = prev.setdefault("sync_info", {})
                    psi["on_wait"] = [extras.pop()]
                # Remaining extras ride single-wait Drains inserted before
                # the instruction (and before its Ldweights, if any).
                ipos = len(out)
                if (
                    prev is not None
                    and prev.get("opcode") == "Ldweights"
                    and prev.get("engine") == ins["engine"]
                ):
                    ipos -= 1
                for extra in extras:
                    out.insert(ipos, mk_drain(ins, extra))
                out.append(ins)
            blk["instructions"] = out
    return json.dumps(d).encode()


def build_nc(bpc=BPC):
    nc = bass.Bass()

    ft_d = nc.dram_tensor("ft", [bpc, KT, 128, N], BF16, kind="ExternalInput")
    posf_d = nc.dram_tensor("posf", [bpc, 1, N], F32, kind="ExternalInput")
    pospack_d = nc.dram_tensor("pospack", [bpc, 128, MT], F32, kind="ExternalInput")
    vmask_d = nc.dram_tensor("vmask", [128, MT], F32, kind="ExternalInput")
    out_d = nc.dram_tensor("out", [1, 1], F32, kind="ExternalOutput")

    with tile.TileContext(nc) as tc:
        with (
            tc.tile_pool(name="ftp", bufs=bpc) as ftp,          # raw fT (resident)
            tc.tile_pool(name="ftnp", bufs=2) as ftnp,          # normalized fT
            tc.tile_pool(name="sqp", bufs=7) as sqp,            # squares
            tc.tile_pool(name="bcp", bufs=2) as bcp,            # broadcast tiles
            tc.tile_pool(name="smallp", bufs=3) as smallp,      # small per-batch
            tc.tile_pool(name="scrp", bufs=2) as scrp,          # STT scratch out
            tc.tile_pool(name="singles", bufs=1) as singles,
            tc.tile_pool(name="psb", bufs=2, space=bass.MemorySpace.PSUM) as psb,
            tc.tile_pool(name="psn", bufs=2, space=bass.MemorySpace.PSUM) as psn,
            tc.tile_pool(name="drp", bufs=2, space="DRAM") as drp,
        ):
            ones_bf = singles.tile([128, 1], BF16)
            nc.vector.memset(ones_bf, 1.0)
            ones_f32 = singles.tile([128, 1], F32)
            nc.vector.memset(ones_f32, 1.0)
            vmask_t = singles.tile([128, MT], F32)
            nc.sync.dma_start(out=vmask_t, in_=vmask_d[:])
            eps_t = singles.tile([1, 1], F32)
            nc.vector.memset(eps_t, 1e-24)
            acc = singles.tile([128, MT], F32)
            nc.vector.memset(acc, 0.0)

            # ================= phase 1: norms for all batches =============
            ft_all = []
            r_drams = []
            for b in range(bpc):
                ft_t = ftp.tile([128, KT, N], BF16, tag="ft", name=f"ft{b}")
                nc.sync.dma_start(
                    out=ft_t, in_=ft_d[b].rearrange("k p n -> p k n")
                )
                ft_all.append(ft_t)

                psum_n = psn.tile([1, 640], F32, tag="pn")
                for k in range(KT):
                    sq_k = sqp.tile([128, N], BF16, tag="sq")
                    nc.scalar.activation(sq_k, ft_t[:, k, :], AF.Square)
                    nc.tensor.matmul(
                        psum_n[:, 0:512], ones_bf, sq_k[:, 0:512],
                        start=(k == 0), stop=(k == KT - 1),
                    )
                    nc.tensor.matmul(
                        psum_n[:, 512:N], ones_bf, sq_k[:, 512:N],
                        start=(k == 0), stop=(k == KT - 1),
                    )
                ln_row = smallp.tile([1, N], F32, tag="lnr")
                nc.scalar.activation(ln_row, psum_n[:, 0:N], AF.Ln, bias=eps_t)
                r_row = smallp.tile([1, N], BF16, tag="rr")
                nc.scalar.activation(r_row, ln_row, AF.Exp, scale=-0.5)
                r_dram = drp.tile([1, N], BF16, tag="rd", name=f"rd{b}", bufs=bpc)
                nc.sync.dma_start(out=r_dram, in_=r_row)
                r_drams.append(r_dram)

            # ================= phase 2: sim + loss per batch ==============
            for b in range(bpc):
                ft_t = ft_all[b]
                r_bc = bcp.tile([128, N], BF16, tag="rbc")
                nc.sync.dma_start(
                    out=r_bc, in_=r_drams[b][:].to_broadcast([128, N])
                )
                pos_bc = bcp.tile([128, N], F32, tag="pbc")
                nc.sync.dma_start(
                    out=pos_bc, in_=posf_d[b].to_broadcast([128, N])
                )
                pos_pack = smallp.tile([128, MT], F32, tag="ppk")
                nc.sync.dma_start(out=pos_pack, in_=pospack_d[b])

                # normalize + S accumulation (S chunk -> column N, streamed
                # by the sim matmul so t = f_i . S lands in psum col N free)
                ftn_t = ftnp.tile([128, KT, N + 2], BF16)
                for k in range(KT):
                    nc.vector.scalar_tensor_tensor(
                        out=ftn_t[:, k, 0:N],
                        in0=ft_t[:, k, :],
                        scalar=1.0,
                        in1=r_bc,
                        op0=ALU.mult,
                        op1=ALU.mult,
                        accum_out=ftn_t[:, k, N : N + 1],
                    )

                # upper-triangular sim: row tile m covers j in [128m, N)
                # plus the S column; the strictly-lower contributions come
                # from column sums of earlier tiles' masked scratch (CC).
                ps_pack = smallp.tile([128, MT], F32, tag="pspk")
                arg_pack = smallp.tile([128, MT], F32, tag="argpk")
                nc.vector.memset(arg_pack, 0.0)
                cc_ps = psn.tile([1, 640], F32, tag="pn")
                for m in range(MT):
                    mm = min(128, N - m * 128)
                    lo = m * 128
                    psum_s = psb.tile([128, 640], F32, tag="ps")
                    regions = []
                    if lo < 512:
                        regions.append((lo, 512))
                    regions.append((512, N + 1))
                    for (j0, j1) in regions:
                        for k in range(KT):
                            nc.tensor.matmul(
                                psum_s[:mm, j0:j1],
                                ftn_t[:, k, lo : lo + mm],
                                ftn_t[:, k, j0:j1],
                                start=(k == 0), stop=(k == KT - 1),
                            )
                    # possum row-part: (pos == pos_i) * sim over j >= 128m
                    scr = scrp.tile([128, N], BF16, tag="scr")
                    nc.vector.scalar_tensor_tensor(
                        out=scr[:mm, 0 : N - lo],
                        in0=pos_bc[:mm, lo:N],
                        scalar=pos_pack[:mm, m : m + 1],
                        in1=psum_s[:mm, lo:N],
                        op0=ALU.is_equal,
                        op1=ALU.mult,
                        accum_out=ps_pack[:mm, m : m + 1],
                    )
                    # arg = t - 2*ps_rowpart
                    nc.vector.scalar_tensor_tensor(
                        out=arg_pack[:mm, m : m + 1],
                        in0=ps_pack[:mm, m : m + 1],
                        scalar=-2.0,
                        in1=psum_s[:mm, N : N + 1],
                        op0=ALU.mult,
                        op1=ALU.add,
                    )
                    # column sums of the strict-upper masked values feed the
                    # lower-triangle row sums (sim symmetry): CC[j] += sum_i
                    if m < MT - 1:
                        g0 = lo + 128  # strict upper starts one block later
                        if g0 < 512:
                            nc.tensor.matmul(
                                cc_ps[:, g0:512],
                                ones_bf[:mm, :],
                                scr[:mm, 128 : 512 - lo],
                                start=(m == 0), stop=(m == 2),
                                skip_group_check=True,
                            )
                        nc.tensor.matmul(
                            cc_ps[:, 512:N],
                            ones_bf[:mm, :],
                            scr[:mm, 512 - lo : N - lo],
                            start=(m == 0), stop=(m == MT - 2),
                            skip_group_check=True,
                        )

                # CC -> [128, MT] pack layout via DRAM round-trip
                # (512-wide, zero-padded past N-128 so the repack never
                # reads out of bounds)
                cc_row = smallp.tile([1, 512], F32, tag="ccr")
                nc.vector.memset(cc_row[:, N - 128 :], 0.0)
                nc.scalar.copy(cc_row[:, 0 : N - 128], cc_ps[:, 128:N])
                cc_dram = drp.tile([1, 512], F32, tag="ccd")
                nc.sync.dma_start(out=cc_dram, in_=cc_row)
                cc_pack = smallp.tile([128, MT], F32, tag="ccp")
                nc.vector.memset(cc_pack[:, 0:1], 0.0)
                nc.sync.dma_start(
                    out=cc_pack[:, 1:MT],
                    in_=cc_dram.rearrange("o (m p) -> o p m", p=128)[0],
                )
                # arg -= 2*CC ; y = (arg + 1)/T
                arg2 = smallp.tile([128, MT], F32, tag="arg2")
                nc.vector.scalar_tensor_tensor(
                    out=arg2, in0=cc_pack, scalar=-2.0, in1=arg_pack,
                    op0=ALU.mult, op1=ALU.add,
                )
                y = smallp.tile([128, MT], F32, tag="y")
                nc.vector.tensor_scalar(
                    out=y, in0=arg2, scalar1=1.0, scalar2=1.0 / TEMP,
                    op0=ALU.add, op1=ALU.mult,
                )
                ab = smallp.tile([128, MT], F32, tag="ab")
                nc.scalar.activation(ab, y, AF.Abs)
                ex = smallp.tile([128, MT], F32, tag="ex")
                nc.scalar.activation(ex, ab, AF.Exp, scale=-1.0)
                ln1p = smallp.tile([128, MT], F32, tag="ln1p")
                nc.scalar.activation(ln1p, ex, AF.Ln, bias=1.0)
                mx = smallp.tile([128, MT], F32, tag="mx")
                nc.scalar.activation(mx, y, AF.Relu)
                sp = smallp.tile([128, MT], F32, tag="sp")
                nc.vector.tensor_add(sp, mx, ln1p)
                spm = smallp.tile([128, MT], F32, tag="spm")
                nc.vector.tensor_mul(spm, sp, vmask_t)
                nc.vector.tensor_add(acc, acc, spm)

            # ---- final: sum acc over all entries -> scalar ----
            red = singles.tile([128, 1], F32)
            nc.vector.reduce_sum(red, acc, axis=mybir.AxisListType.X)
            psum_f = psn.tile([1, 1], F32, tag="pn")
            nc.tensor.matmul(psum_f, ones_f32, red)
            out_sb = singles.tile([1, 1], F32)
            nc.scalar.copy(out_sb, psum_f)
            nc.sync.dma_start(out=out_d[:], in_=out_sb)

    nc.finalize()
    fixed = _legalize_sync_json(bytes(nc.to_json_bytes()))
    nc.to_json_bytes = lambda: fixed  # instance override: walrus-legal BIR
    return nc


def _prep_inputs(features, positions, bpc_total=B):
    feats = np.asarray(features, dtype=np.float32).reshape(B, N, D)
    pos = np.asarray(positions).astype(np.float32)  # values < 2^24, exact
    fT = np.ascontiguousarray(feats.transpose(0, 2, 1))  # [B, D, N]
    fT = fT.reshape(B, KT, 128, N).astype(ml_dtypes.bfloat16)
    pos_pack = np.full((B, 128, MT), -1.0, dtype=np.float32)
    for m in range(MT):
        lo = m * 128
        hi = min(N, lo + 128)
        pos_pack[:, : hi - lo, m] = pos[:, lo:hi]
    vmask = np.zeros((128, MT), dtype=np.float32)
    for m in range(MT):
        lo = m * 128
        hi = min(N, lo + 128)
        vmask[: hi - lo, m] = 1.0
    return fT, pos.reshape(B, 1, N), pos_pack, vmask


def _install_ntff_hook_shim():
    """This image's boot skipped installing the axon NTFF profile hook
    (no antenv.axon_hooks module). Recreate it so trace=True works."""
    import sys as _sys
    import types as _types

    if "antenv.axon_hooks" in _sys.modules:
        return
    try:
        from trn_agent_boot.trn_boot import _ntff_profile_via_ctypes

        hook = _ntff_profile_via_ctypes("/opt/axon/libaxon_pjrt.so")
    except Exception:
        return
    import antenv as _antenv

    mod = _types.ModuleType("antenv.axon_hooks")
    mod.get_axon_ntff_profile_hook = lambda: hook
    mod.set_axon_ntff_profile_hook = lambda h: None
    _sys.modules["antenv.axon_hooks"] = mod
    _antenv.axon_hooks = mod


_install_ntff_hook_shim()

_NC_CACHE = {}
LAST_RESULTS = None  # BassKernelResults of the most recent run (for profiling)


def kernel(features, positions, _trace=False):
    global LAST_RESULTS
    fT, posf, pos_pack, vmask = _prep_inputs(features, positions)
    if BPC not in _NC_CACHE:
        _NC_CACHE[BPC] = build_nc(BPC)
    nc = _NC_CACHE[BPC]
    in_maps = []
    for c in range(NCORES):
        s = slice(c * BPC, (c + 1) * BPC)
        in_maps.append(
            {
                "ft": np.ascontiguousarray(fT[s]),
                "posf": np.ascontiguousarray(posf[s]),
                "pospack": np.ascontiguousarray(pos_pack[s]),
                "vmask": vmask,
            }
        )
    res = run_bass_kernel_spmd(
        nc, in_maps, core_ids=list(range(NCORES)), trace=_trace
    )
    LAST_RESULTS = res
    total = sum(float(r["out"][0, 0]) for r in res.results)
    return np.float32(total / (B * N))


# revision 23
# speedup vs baseline: 1.1878x; 1.0446x over previous
"""Trainium2 Bass kernel for ContrastivePuzzleLoss.

Reference math (per batch b):
    f = features / max(||features||_2, 1e-12)           (L2 norm over D)
    sim = (f @ f.T) / T,  off-diag only
    pos_mask[i,j] = (pos_i == pos_j), off-diag only
    pos_s = sum_j sim*mask + eps ; neg_s = sum_j sim*(1-mask) + eps
    loss = mean softplus(neg_s - pos_s)

Key identity used here: eps cancels and
    neg_s - pos_s = rowsum_offdiag - 2*possum_offdiag
                  = ((f_i . S) - 2*(sum_j delta_ij f_i.f_j) + 1) / T
with S = sum_j f_j. Per-core work (pure data parallel over batch):
  - load transposed features fT [D,N] (bf16)
  - norms via ACT Square + ones-matmul (PE), r = exp(-0.5*ln(norm^2+1e-24))
  - normalize: one fused DVE scalar_tensor_tensor pass per k-tile
    (out = fT * r_bcast, accum_out = S chunk)
  - sim rows via PE matmul (bf16), with f_i.S folded in as one extra
    psum column (matvec against S)
  - possum in ONE fused DVE pass per row tile:
    (pos_bcast == pos_i) * sim, accumulated along the row
  - softplus via ACT: relu(x) + ln(1 + exp(-|x|))
  - per-core scalar = sum of losses; host sums cores and divides.
"""

import json

import numpy as np
import ml_dtypes

import concourse.bass as bass
import concourse.tile as tile
import concourse.mybir as mybir
from concourse.bass_utils import run_bass_kernel_spmd

B, N, D = 64, 576, 768
NCORES = 8
BPC = B // NCORES          # batches per core
KT = D // 128              # 6 contraction tiles
MT = (N + 127) // 128      # 5 row tiles (last has 64 rows)
TEMP = 0.07

F32 = mybir.dt.float32
F32R = mybir.dt.float32r
BF16 = mybir.dt.bfloat16
AF = mybir.ActivationFunctionType
ALU = mybir.AluOpType


def _legalize_sync_json(raw: bytes) -> bytes:
    """The hardware ISA has ONE sync-wait slot per instruction, and this
    walrus build refuses multi-wait instructions ("Too many sync wait
    commands"). Split extra waits onto injected single-wait Drain
    instructions on the same engine, preceding the original."""
    d = json.loads(raw)
    nid = [0]

    def mk_drain(ins, wait):
        nid[0] += 1
        return {
            "debug": ins.get("debug", 0),
            "engine": ins["engine"],
            "name": f"I-WSPLIT-{nid[0]}",
            "opcode": "Drain",
            "ins": [],
            "outs": [],
            "sync_info": {"on_wait": [wait], "on_update": []},
        }

    for fn in d["functions"]:
        for blk in fn["blocks"]:
            out = []
            for ins in blk["instructions"]:
                si = ins.get("sync_info") or {}
                w = si.get("on_wait") or []
                if len(w) <= 1:
                    out.append(ins)
                    continue
                extras = w[:-1]
                si["on_wait"] = [w[-1]]
                # A PE Matmult is normally preceded by its Ldweights with a
                # free wait slot — park one wait there (no pipeline flush).
                prev = out[-1] if out else None
                if (
                    ins["opcode"] == "Matmult"
                    and prev is not None
                    and prev.get("opcode") == "Ldweights"
                    and prev.get("engine") == ins["engine"]
                    and not ((prev.get("sync_info") or {}).get("on_wait") or [])
                ):
                    psi = prev.setdefault("sync_info", {})
                    psi["on_wait"] = [extras.pop()]
                # Remaining extras ride single-wait Drains inserted before
                # the instruction (and before its Ldweights, if any).
                ipos = len(out)
                if (
                    prev is not None
                    and prev.get("opcode") == "Ldweights"
                    and prev.get("engine") == ins["engine"]
                ):
                    ipos -= 1
                for extra in extras:
                    out.insert(ipos, mk_drain(ins, extra))
                out.append(ins)
            blk["instructions"] = out
    return json.dumps(d).encode()


def build_nc(bpc=BPC):
    nc = bass.Bass()

    ft_d = nc.dram_tensor("ft", [bpc, KT, 128, N], BF16, kind="ExternalInput")
    posf_d = nc.dram_tensor("posf", [bpc, 1, N], F32, kind="ExternalInput")
    pospack_d = nc.dram_tensor("pospack", [bpc, 128, MT], F32, kind="ExternalInput")
    vmask_d = nc.dram_tensor("vmask", [128, MT], F32, kind="ExternalInput")
    out_d = nc.dram_tensor("out", [1, 1], F32, kind="ExternalOutput")

    with tile.TileContext(nc) as tc:
        with (
            tc.tile_pool(name="ftp", bufs=bpc) as ftp,          # raw fT (resident)
            tc.tile_pool(name="ftnp", bufs=2) as ftnp,          # normalized fT
            tc.tile_pool(name="sqp", bufs=7) as sqp,            # squares
            tc.tile_pool(name="bcp", bufs=2) as bcp,            # broadcast tiles
            tc.tile_pool(name="smallp", bufs=3) as smallp,      # small per-batch
            tc.tile_pool(name="scrp", bufs=2) as scrp,          # STT scratch out
            tc.tile_pool(name="singles", bufs=1) as singles,
            tc.tile_pool(name="psb", bufs=3, space=bass.MemorySpace.PSUM) as psb,
            tc.tile_pool(name="psn", bufs=1, space=bass.MemorySpace.PSUM) as psn,
            tc.tile_pool(name="drp", bufs=2, space="DRAM") as drp,
        ):
            ones_bf = singles.tile([128, 1], BF16)
            nc.vector.memset(ones_bf, 1.0)
            ones_f32 = singles.tile([128, 1], F32)
            nc.vector.memset(ones_f32, 1.0)
            vmask_t = singles.tile([128, MT], F32)
            nc.sync.dma_start(out=vmask_t, in_=vmask_d[:])
            eps_t = singles.tile([1, 1], F32)
            nc.vector.memset(eps_t, 1e-24)
            acc = singles.tile([128, MT], F32)
            nc.vector.memset(acc, 0.0)

            # ================= phase 1: norms for all batches =============
            ft_all = []
            r_drams = []
            for b in range(bpc):
                ft_t = ftp.tile([128, KT, N], BF16, tag="ft", name=f"ft{b}")
                nc.scalar.dma_start(
                    out=ft_t, in_=ft_d[b].rearrange("k p n -> p k n")
                )
                ft_all.append(ft_t)

                psum_n = psn.tile([1, 640], F32, tag="pn")
                for k in range(KT):
                    sq_k = sqp.tile([128, N], BF16, tag="sq")
                    nc.scalar.activation(sq_k, ft_t[:, k, :], AF.Square)
                    nc.tensor.matmul(
                        psum_n[:, 0:512], ones_bf, sq_k[:, 0:512],
                        start=(k == 0), stop=(k == KT - 1),
                    )
                    nc.tensor.matmul(
                        psum_n[:, 512:N], ones_bf, sq_k[:, 512:N],
                        start=(k == 0), stop=(k == KT - 1),
                    )
                ln_row = smallp.tile([1, N], F32, tag="lnr")
                nc.scalar.activation(ln_row, psum_n[:, 0:N], AF.Ln, bias=eps_t)
                r_row = smallp.tile([1, N], BF16, tag="rr")
                nc.scalar.activation(r_row, ln_row, AF.Exp, scale=-0.5)
                r_dram = drp.tile([1, N], BF16, tag="rd", name=f"rd{b}", bufs=bpc)
                nc.sync.dma_start(out=r_dram, in_=r_row)
                r_drams.append(r_dram)

            # ================= phase 2: sim + loss per batch ==============
            for b in range(bpc):
                ft_t = ft_all[b]
                r_bc = bcp.tile([128, N], BF16, tag="rbc")
                nc.sync.dma_start(
                    out=r_bc, in_=r_drams[b][:].to_broadcast([128, N])
                )
                pos_bc = bcp.tile([128, N], F32, tag="pbc")
                nc.sync.dma_start(
                    out=pos_bc, in_=posf_d[b].to_broadcast([128, N])
                )
                pos_pack = smallp.tile([128, MT], F32, tag="ppk")
                nc.sync.dma_start(out=pos_pack, in_=pospack_d[b])

                # normalize + S accumulation (S chunk -> column N, streamed
                # by the sim matmul so t = f_i . S lands in psum col N free)
                ftn_t = ftnp.tile([128, KT, N + 2], BF16)
                for k in range(KT):
                    nc.vector.scalar_tensor_tensor(
                        out=ftn_t[:, k, 0:N],
                        in0=ft_t[:, k, :],
                        scalar=1.0,
                        in1=r_bc,
                        op0=ALU.mult,
                        op1=ALU.mult,
                        accum_out=ftn_t[:, k, N : N + 1],
                    )

                # upper-triangular sim: row tile m covers j in [128m, N)
                # plus the S column; the strictly-lower contributions come
                # from column sums of earlier tiles' masked scratch (CC).
                ps_pack = smallp.tile([128, MT], F32, tag="pspk")
                arg_pack = smallp.tile([128, MT], F32, tag="argpk")
                nc.vector.memset(arg_pack, 0.0)
                cc_ps = psn.tile([1, 640], F32, tag="pn")
                for m in range(MT):
                    mm = min(128, N - m * 128)
                    lo = m * 128
                    psum_s = psb.tile([128, 640], F32, tag="ps")
                    regions = []
                    if lo < 512:
                        regions.append((lo, 512))
                    regions.append((512, N + 1))
                    for (j0, j1) in regions:
                        for k in range(KT):
                            nc.tensor.matmul(
                                psum_s[:mm, j0:j1],
                                ftn_t[:, k, lo : lo + mm],
                                ftn_t[:, k, j0:j1],
                                start=(k == 0), stop=(k == KT - 1),
                            )
                    # possum row-part: (pos == pos_i) * sim over j >= 128m
                    scr = scrp.tile([128, N], BF16, tag="scr")
                    nc.vector.scalar_tensor_tensor(
                        out=scr[:mm, 0 : N - lo],
                        in0=pos_bc[:mm, lo:N],
                        scalar=pos_pack[:mm, m : m + 1],
                        in1=psum_s[:mm, lo:N],
                        op0=ALU.is_equal,
                        op1=ALU.mult,
                        accum_out=ps_pack[:mm, m : m + 1],
                    )
                    # arg = t - 2*ps_rowpart
                    nc.vector.scalar_tensor_tensor(
                        out=arg_pack[:mm, m : m + 1],
                        in0=ps_pack[:mm, m : m + 1],
                        scalar=-2.0,
                        in1=psum_s[:mm, N : N + 1],
                        op0=ALU.mult,
                        op1=ALU.add,
                    )
                    # column sums of the strict-upper masked values feed the
                    # lower-triangle row sums (sim symmetry): CC[j] += sum_i
                    if m < MT - 1:
                        g0 = lo + 128  # strict upper starts one block later
                        if g0 < 512:
                            nc.tensor.matmul(
                                cc_ps[:, g0:512],
                                ones_bf[:mm, :],
                                scr[:mm, 128 : 512 - lo],
                                start=(m == 0), stop=(m == 2),
                                skip_group_check=True,
                            )
                        nc.tensor.matmul(
                            cc_ps[:, 512:N],
                            ones_bf[:mm, :],
                            scr[:mm, 512 - lo : N - lo],
                            start=(m == 0), stop=(m == MT - 2),
                            skip_group_check=True,
                        )

                # CC -> [128, MT] pack layout via DRAM round-trip
                # (512-wide, zero-padded past N-128 so the repack never
                # reads out of bounds)
                cc_row = smallp.tile([1, 512], F32, tag="ccr")
                nc.vector.memset(cc_row[:, N - 128 :], 0.0)
                nc.scalar.copy(cc_row[:, 0 : N - 128], cc_ps[:, 128:N])
                cc_dram = drp.tile([1, 512], F32, tag="ccd")
                nc.sync.dma_start(out=cc_dram, in_=cc_row)
                cc_pack = smallp.tile([128, MT], F32, tag="ccp")
                nc.vector.memset(cc_pack[:, 0:1], 0.0)
                nc.sync.dma_start(
                    out=cc_pack[:, 1:MT],
                    in_=cc_dram.rearrange("o (m p) -> o p m", p=128)[0],
                )
                # arg -= 2*CC ; y = (arg + 1)/T
                arg2 = smallp.tile([128, MT], F32, tag="arg2")
                nc.vector.scalar_tensor_tensor(
                    out=arg2, in0=cc_pack, scalar=-2.0, in1=arg_pack,
                    op0=ALU.mult, op1=ALU.add,
                )
                y = smallp.tile([128, MT], F32, tag="y")
                nc.vector.tensor_scalar(
                    out=y, in0=arg2, scalar1=1.0, scalar2=1.0 / TEMP,
                    op0=ALU.add, op1=ALU.mult,
                )
                ab = smallp.tile([128, MT], F32, tag="ab")
                nc.scalar.activation(ab, y, AF.Abs)
                ex = smallp.tile([128, MT], F32, tag="ex")
                nc.scalar.activation(ex, ab, AF.Exp, scale=-1.0)
                ln1p = smallp.tile([128, MT], F32, tag="ln1p")
                nc.scalar.activation(ln1p, ex, AF.Ln, bias=1.0)
                mx = smallp.tile([128, MT], F32, tag="mx")
                nc.scalar.activation(mx, y, AF.Relu)
                sp = smallp.tile([128, MT], F32, tag="sp")
                nc.vector.tensor_add(sp, mx, ln1p)
                spm = smallp.tile([128, MT], F32, tag="spm")
                nc.vector.tensor_mul(spm, sp, vmask_t)
                nc.vector.tensor_add(acc, acc, spm)

            # ---- final: sum acc over all entries -> scalar ----
            red = singles.tile([128, 1], F32)
            nc.vector.reduce_sum(red, acc, axis=mybir.AxisListType.X)
            psum_f = psn.tile([1, 1], F32, tag="pn")
            nc.tensor.matmul(psum_f, ones_f32, red)
            out_sb = singles.tile([1, 1], F32)
            nc.scalar.copy(out_sb, psum_f)
            nc.sync.dma_start(out=out_d[:], in_=out_sb)

    nc.finalize()
    fixed = _legalize_sync_json(bytes(nc.to_json_bytes()))
    nc.to_json_bytes = lambda: fixed  # instance override: walrus-legal BIR
    return nc


def _prep_inputs(features, positions, bpc_total=B):
    feats = np.asarray(features, dtype=np.float32).reshape(B, N, D)
    pos = np.asarray(positions).astype(np.float32)  # values < 2^24, exact
    fT = np.ascontiguousarray(feats.transpose(0, 2, 1))  # [B, D, N]
    fT = fT.reshape(B, KT, 128, N).astype(ml_dtypes.bfloat16)
    pos_pack = np.full((B, 128, MT), -1.0, dtype=np.float32)
    for m in range(MT):
        lo = m * 128
        hi = min(N, lo + 128)
        pos_pack[:, : hi - lo, m] = pos[:, lo:hi]
    vmask = np.zeros((128, MT), dtype=np.float32)
    for m in range(MT):
        lo = m * 128
        hi = min(N, lo + 128)
        vmask[: hi - lo, m] = 1.0
    return fT, pos.reshape(B, 1, N), pos_pack, vmask


def _install_ntff_hook_shim():
    """This image's boot skipped installing the axon NTFF profile hook
    (no antenv.axon_hooks module). Recreate it so trace=True works."""
    import sys as _sys
    import types as _types

    if "antenv.axon_hooks" in _sys.modules:
        return
    try:
        from trn_agent_boot.trn_boot import _ntff_profile_via_ctypes

        hook = _ntff_profile_via_ctypes("/opt/axon/libaxon_pjrt.so")
    except Exception:
        return
    import antenv as _antenv

    mod = _types.ModuleType("antenv.axon_hooks")
    mod.get_axon_ntff_profile_hook = lambda: hook
    mod.set_axon_ntff_profile_hook = lambda h: None
    _sys.modules["antenv.axon_hooks"] = mod
    _antenv.axon_hooks = mod


_install_ntff_hook_shim()

_NC_CACHE = {}
LAST_RESULTS = None  # BassKernelResults of the most recent run (for profiling)


def kernel(features, positions, _trace=False):
    global LAST_RESULTS
    fT, posf, pos_pack, vmask = _prep_inputs(features, positions)
    if BPC not in _NC_CACHE:
        _NC_CACHE[BPC] = build_nc(BPC)
    nc = _NC_CACHE[BPC]
    in_maps = []
    for c in range(NCORES):
        s = slice(c * BPC, (c + 1) * BPC)
        in_maps.append(
            {
                "ft": np.ascontiguousarray(fT[s]),
                "posf": np.ascontiguousarray(posf[s]),
                "pospack": np.ascontiguousarray(pos_pack[s]),
                "vmask": vmask,
            }
        )
    res = run_bass_kernel_spmd(
        nc, in_maps, core_ids=list(range(NCORES)), trace=_trace
    )
    LAST_RESULTS = res
    total = sum(float(r["out"][0, 0]) for r in res.results)
    return np.float32(total / (B * N))


# revision 24
# speedup vs baseline: 1.1900x; 1.0018x over previous
"""Trainium2 Bass kernel for ContrastivePuzzleLoss.

Reference math (per batch b):
    f = features / max(||features||_2, 1e-12)           (L2 norm over D)
    sim = (f @ f.T) / T,  off-diag only
    pos_mask[i,j] = (pos_i == pos_j), off-diag only
    pos_s = sum_j sim*mask + eps ; neg_s = sum_j sim*(1-mask) + eps
    loss = mean softplus(neg_s - pos_s)

Key identity used here: eps cancels and
    neg_s - pos_s = rowsum_offdiag - 2*possum_offdiag
                  = ((f_i . S) - 2*(sum_j delta_ij f_i.f_j) + 1) / T
with S = sum_j f_j. Per-core work (pure data parallel over batch):
  - load transposed features fT [D,N] (bf16)
  - norms via ACT Square + ones-matmul (PE), r = exp(-0.5*ln(norm^2+1e-24))
  - normalize: one fused DVE scalar_tensor_tensor pass per k-tile
    (out = fT * r_bcast, accum_out = S chunk)
  - sim rows via PE matmul (bf16), with f_i.S folded in as one extra
    psum column (matvec against S)
  - possum in ONE fused DVE pass per row tile:
    (pos_bcast == pos_i) * sim, accumulated along the row
  - softplus via ACT: relu(x) + ln(1 + exp(-|x|))
  - per-core scalar = sum of losses; host sums cores and divides.
"""

import json

import numpy as np
import ml_dtypes

import concourse.bass as bass
import concourse.tile as tile
import concourse.mybir as mybir
from concourse.bass_utils import run_bass_kernel_spmd

B, N, D = 64, 576, 768
NCORES = 8
BPC = B // NCORES          # batches per core
KT = D // 128              # 6 contraction tiles
MT = (N + 127) // 128      # 5 row tiles (last has 64 rows)
TEMP = 0.07

F32 = mybir.dt.float32
F32R = mybir.dt.float32r
BF16 = mybir.dt.bfloat16
AF = mybir.ActivationFunctionType
ALU = mybir.AluOpType


def _legalize_sync_json(raw: bytes) -> bytes:
    """The hardware ISA has ONE sync-wait slot per instruction, and this
    walrus build refuses multi-wait instructions ("Too many sync wait
    commands"). Split extra waits onto injected single-wait Drain
    instructions on the same engine, preceding the original."""
    d = json.loads(raw)
    nid = [0]

    def mk_drain(ins, wait):
        nid[0] += 1
        return {
            "debug": ins.get("debug", 0),
            "engine": ins["engine"],
            "name": f"I-WSPLIT-{nid[0]}",
            "opcode": "Drain",
            "ins": [],
            "outs": [],
            "sync_info": {"on_wait": [wait], "on_update": []},
        }

    for fn in d["functions"]:
        for blk in fn["blocks"]:
            out = []
            for ins in blk["instructions"]:
                si = ins.get("sync_info") or {}
                w = si.get("on_wait") or []
                if len(w) <= 1:
                    out.append(ins)
                    continue
                extras = w[:-1]
                si["on_wait"] = [w[-1]]
                # A PE Matmult is normally preceded by its Ldweights with a
                # free wait slot — park one wait there (no pipeline flush).
                prev = out[-1] if out else None
                if (
                    ins["opcode"] == "Matmult"
                    and prev is not None
                    and prev.get("opcode") == "Ldweights"
                    and prev.get("engine") == ins["engine"]
                    and not ((prev.get("sync_info") or {}).get("on_wait") or [])
                ):
                    psi = prev.setdefault("sync_info", {})
                    psi["on_wait"] = [extras.pop()]
                # Remaining extras ride single-wait Drains inserted before
                # the instruction (and before its Ldweights, if any).
                ipos = len(out)
                if (
                    prev is not None
                    and prev.get("opcode") == "Ldweights"
                    and prev.get("engine") == ins["engine"]
                ):
                    ipos -= 1
                for extra in extras:
                    out.insert(ipos, mk_drain(ins, extra))
                out.append(ins)
            blk["instructions"] = out
    return json.dumps(d).encode()


def build_nc(bpc=BPC):
    nc = bass.Bass()

    ft_d = nc.dram_tensor("ft", [bpc, KT, 128, N], BF16, kind="ExternalInput")
    posf_d = nc.dram_tensor("posf", [bpc, 1, N], F32, kind="ExternalInput")
    pospack_d = nc.dram_tensor("pospack", [bpc, 128, MT], F32, kind="ExternalInput")
    vmask_d = nc.dram_tensor("vmask", [128, MT], F32, kind="ExternalInput")
    out_d = nc.dram_tensor("out", [1, 1], F32, kind="ExternalOutput")

    with tile.TileContext(nc) as tc:
        with (
            tc.tile_pool(name="ftp", bufs=bpc) as ftp,          # raw fT (resident)
            tc.tile_pool(name="ftnp", bufs=2) as ftnp,          # normalized fT
            tc.tile_pool(name="sqp", bufs=7) as sqp,            # squares
            tc.tile_pool(name="bcp", bufs=2) as bcp,            # broadcast tiles
            tc.tile_pool(name="smallp", bufs=3) as smallp,      # small per-batch
            tc.tile_pool(name="scrp", bufs=2) as scrp,          # STT scratch out
            tc.tile_pool(name="singles", bufs=1) as singles,
            tc.tile_pool(name="psb", bufs=3, space=bass.MemorySpace.PSUM) as psb,
            tc.tile_pool(name="psn", bufs=1, space=bass.MemorySpace.PSUM) as psn,
            tc.tile_pool(name="drp", bufs=2, space="DRAM") as drp,
        ):
            ones_bf = singles.tile([128, 1], BF16)
            nc.vector.memset(ones_bf, 1.0)
            ones_f32 = singles.tile([128, 1], F32)
            nc.vector.memset(ones_f32, 1.0)
            vmask_t = singles.tile([128, MT], F32)
            nc.sync.dma_start(out=vmask_t, in_=vmask_d[:])
            eps_t = singles.tile([1, 1], F32)
            nc.vector.memset(eps_t, 1e-24)
            acc = singles.tile([128, MT], F32)
            nc.vector.memset(acc, 0.0)

            # ================= phase 1: norms for all batches =============
            ft_all = []
            r_drams = []
            for b in range(bpc):
                ft_t = ftp.tile([128, KT, N], BF16, tag="ft", name=f"ft{b}")
                nc.scalar.dma_start(
                    out=ft_t, in_=ft_d[b].rearrange("k p n -> p k n")
                )
                ft_all.append(ft_t)

                psum_n = psn.tile([1, 640], F32, tag="pn")
                for k in range(KT):
                    sq_k = sqp.tile([128, N], BF16, tag="sq")
                    nc.scalar.activation(sq_k, ft_t[:, k, :], AF.Square)
                    nc.tensor.matmul(
                        psum_n[:, 0:512], ones_bf, sq_k[:, 0:512],
                        start=(k == 0), stop=(k == KT - 1),
                    )
                    nc.tensor.matmul(
                        psum_n[:, 512:N], ones_bf, sq_k[:, 512:N],
                        start=(k == 0), stop=(k == KT - 1),
                    )
                ln_row = smallp.tile([1, N], F32, tag="lnr")
                nc.scalar.activation(ln_row, psum_n[:, 0:N], AF.Ln, bias=eps_t)
                r_row = smallp.tile([1, N], BF16, tag="rr")
                nc.scalar.activation(r_row, ln_row, AF.Exp, scale=-0.5)
                r_dram = drp.tile([1, N], BF16, tag="rd", name=f"rd{b}", bufs=bpc)
                nc.sync.dma_start(out=r_dram, in_=r_row)
                r_drams.append(r_dram)

            # ================= phase 2: sim + loss per batch ==============
            for b in range(bpc):
                ft_t = ft_all[b]
                r_bc = bcp.tile([128, N], BF16, tag="rbc")
                nc.sync.dma_start(
                    out=r_bc, in_=r_drams[b][:].to_broadcast([128, N])
                )
                pos_bc = bcp.tile([128, N], F32, tag="pbc")
                nc.sync.dma_start(
                    out=pos_bc, in_=posf_d[b].to_broadcast([128, N])
                )
                pos_pack = smallp.tile([128, MT], F32, tag="ppk")
                nc.sync.dma_start(out=pos_pack, in_=pospack_d[b])

                # normalize + S accumulation (S chunk -> column N, streamed
                # by the sim matmul so t = f_i . S lands in psum col N free)
                ftn_t = ftnp.tile([128, KT, N + 2], BF16)
                for k in range(KT):
                    nc.vector.scalar_tensor_tensor(
                        out=ftn_t[:, k, 0:N],
                        in0=ft_t[:, k, :],
                        scalar=1.0,
                        in1=r_bc,
                        op0=ALU.mult,
                        op1=ALU.mult,
                        accum_out=ftn_t[:, k, N : N + 1],
                    )

                # upper-triangular sim: row tile m covers j in [128m, N)
                # plus the S column; the strictly-lower contributions come
                # from column sums of earlier tiles' masked scratch (CC).
                ps_pack = smallp.tile([128, MT], F32, tag="pspk")
                arg_pack = smallp.tile([128, MT], F32, tag="argpk")
                nc.vector.memset(arg_pack, 0.0)
                cc_ps = psn.tile([1, 640], F32, tag="pn")
                for m in range(MT):
                    mm = min(128, N - m * 128)
                    lo = m * 128
                    w = N + 1 - lo  # row width incl. S column, LOCAL coords
                    psum_s = psb.tile([128, 640], F32, tag="ps")
                    # local region(s): [0:w] fits one bank for m>=1
                    regions = [(0, min(w, 512))]
                    if w > 512:
                        regions.append((512, w))
                    for (j0, j1) in regions:
                        for k in range(KT):
                            nc.tensor.matmul(
                                psum_s[:mm, j0:j1],
                                ftn_t[:, k, lo : lo + mm],
                                ftn_t[:, k, lo + j0 : lo + j1],
                                start=(k == 0), stop=(k == KT - 1),
                            )
                    # possum row-part: (pos == pos_i) * sim over j >= 128m
                    scr = scrp.tile([128, N], BF16, tag="scr")
                    nc.vector.scalar_tensor_tensor(
                        out=scr[:mm, 0 : N - lo],
                        in0=pos_bc[:mm, lo:N],
                        scalar=pos_pack[:mm, m : m + 1],
                        in1=psum_s[:mm, 0 : N - lo],
                        op0=ALU.is_equal,
                        op1=ALU.mult,
                        accum_out=ps_pack[:mm, m : m + 1],
                    )
                    # arg = t - 2*ps_rowpart  (t in local col N-lo)
                    nc.vector.scalar_tensor_tensor(
                        out=arg_pack[:mm, m : m + 1],
                        in0=ps_pack[:mm, m : m + 1],
                        scalar=-2.0,
                        in1=psum_s[:mm, N - lo : N + 1 - lo],
                        op0=ALU.mult,
                        op1=ALU.add,
                    )
                    # column sums of the strict-upper masked values feed the
                    # lower-triangle row sums (sim symmetry): CC[j] += sum_i
                    if m < MT - 1:
                        g0 = lo + 128  # strict upper starts one block later
                        if g0 < 512:
                            nc.tensor.matmul(
                                cc_ps[:, g0:512],
                                ones_bf[:mm, :],
                                scr[:mm, 128 : 512 - lo],
                                start=(m == 0), stop=(m == 2),
                                skip_group_check=True,
                            )
                        nc.tensor.matmul(
                            cc_ps[:, 512:N],
                            ones_bf[:mm, :],
                            scr[:mm, 512 - lo : N - lo],
                            start=(m == 0), stop=(m == MT - 2),
                            skip_group_check=True,
                        )

                # CC -> [128, MT] pack layout via DRAM round-trip
                # (512-wide, zero-padded past N-128 so the repack never
                # reads out of bounds)
                cc_row = smallp.tile([1, 512], F32, tag="ccr")
                nc.vector.memset(cc_row[:, N - 128 :], 0.0)
                nc.scalar.copy(cc_row[:, 0 : N - 128], cc_ps[:, 128:N])
                cc_dram = drp.tile([1, 512], F32, tag="ccd")
                nc.sync.dma_start(out=cc_dram, in_=cc_row)
                cc_pack = smallp.tile([128, MT], F32, tag="ccp")
                nc.vector.memset(cc_pack[:, 0:1], 0.0)
                nc.sync.dma_start(
                    out=cc_pack[:, 1:MT],
                    in_=cc_dram.rearrange("o (m p) -> o p m", p=128)[0],
                )
                # arg -= 2*CC ; y = (arg + 1)/T
                arg2 = smallp.tile([128, MT], F32, tag="arg2")
                nc.vector.scalar_tensor_tensor(
                    out=arg2, in0=cc_pack, scalar=-2.0, in1=arg_pack,
                    op0=ALU.mult, op1=ALU.add,
                )
                y = smallp.tile([128, MT], F32, tag="y")
                nc.vector.tensor_scalar(
                    out=y, in0=arg2, scalar1=1.0, scalar2=1.0 / TEMP,
                    op0=ALU.add, op1=ALU.mult,
                )
                ab = smallp.tile([128, MT], F32, tag="ab")
                nc.scalar.activation(ab, y, AF.Abs)
                ex = smallp.tile([128, MT], F32, tag="ex")
                nc.scalar.activation(ex, ab, AF.Exp, scale=-1.0)
                ln1p = smallp.tile([128, MT], F32, tag="ln1p")
                nc.scalar.activation(ln1p, ex, AF.Ln, bias=1.0)
                mx = smallp.tile([128, MT], F32, tag="mx")
                nc.scalar.activation(mx, y, AF.Relu)
                sp = smallp.tile([128, MT], F32, tag="sp")
                nc.vector.tensor_add(sp, mx, ln1p)
                spm = smallp.tile([128, MT], F32, tag="spm")
                nc.vector.tensor_mul(spm, sp, vmask_t)
                nc.vector.tensor_add(acc, acc, spm)

            # ---- final: sum acc over all entries -> scalar ----
            red = singles.tile([128, 1], F32)
            nc.vector.reduce_sum(red, acc, axis=mybir.AxisListType.X)
            psum_f = psn.tile([1, 1], F32, tag="pn")
            nc.tensor.matmul(psum_f, ones_f32, red)
            out_sb = singles.tile([1, 1], F32)
            nc.scalar.copy(out_sb, psum_f)
            nc.sync.dma_start(out=out_d[:], in_=out_sb)

    nc.finalize()
    fixed = _legalize_sync_json(bytes(nc.to_json_bytes()))
    nc.to_json_bytes = lambda: fixed  # instance override: walrus-legal BIR
    return nc


def _prep_inputs(features, positions, bpc_total=B):
    feats = np.asarray(features, dtype=np.float32).reshape(B, N, D)
    pos = np.asarray(positions).astype(np.float32)  # values < 2^24, exact
    fT = np.ascontiguousarray(feats.transpose(0, 2, 1))  # [B, D, N]
    fT = fT.reshape(B, KT, 128, N).astype(ml_dtypes.bfloat16)
    pos_pack = np.full((B, 128, MT), -1.0, dtype=np.float32)
    for m in range(MT):
        lo = m * 128
        hi = min(N, lo + 128)
        pos_pack[:, : hi - lo, m] = pos[:, lo:hi]
    vmask = np.zeros((128, MT), dtype=np.float32)
    for m in range(MT):
        lo = m * 128
        hi = min(N, lo + 128)
        vmask[: hi - lo, m] = 1.0
    return fT, pos.reshape(B, 1, N), pos_pack, vmask


def _install_ntff_hook_shim():
    """This image's boot skipped installing the axon NTFF profile hook
    (no antenv.axon_hooks module). Recreate it so trace=True works."""
    import sys as _sys
    import types as _types

    if "antenv.axon_hooks" in _sys.modules:
        return
    try:
        from trn_agent_boot.trn_boot import _ntff_profile_via_ctypes

        hook = _ntff_profile_via_ctypes("/opt/axon/libaxon_pjrt.so")
    except Exception:
        return
    import antenv as _antenv

    mod = _types.ModuleType("antenv.axon_hooks")
    mod.get_axon_ntff_profile_hook = lambda: hook
    mod.set_axon_ntff_profile_hook = lambda h: None
    _sys.modules["antenv.axon_hooks"] = mod
    _antenv.axon_hooks = mod


_install_ntff_hook_shim()

_NC_CACHE = {}
LAST_RESULTS = None  # BassKernelResults of the most recent run (for profiling)


def kernel(features, positions, _trace=False):
    global LAST_RESULTS
    fT, posf, pos_pack, vmask = _prep_inputs(features, positions)
    if BPC not in _NC_CACHE:
        _NC_CACHE[BPC] = build_nc(BPC)
    nc = _NC_CACHE[BPC]
    in_maps = []
    for c in range(NCORES):
        s = slice(c * BPC, (c + 1) * BPC)
        in_maps.append(
            {
                "ft": np.ascontiguousarray(fT[s]),
                "posf": np.ascontiguousarray(posf[s]),
                "pospack": np.ascontiguousarray(pos_pack[s]),
                "vmask": vmask,
            }
        )
    res = run_bass_kernel_spmd(
        nc, in_maps, core_ids=list(range(NCORES)), trace=_trace
    )
    LAST_RESULTS = res
    total = sum(float(r["out"][0, 0]) for r in res.results)
    return np.float32(total / (B * N))
